# revision 3
# baseline (speedup 1.0000x reference)
"""Trainium2 Bass kernel for ColorAttentionModule (histogram binning + 1x1 convs).

Data-parallel over batch: 8 cores x 2 batches; per core 128 partition rows
(2 batches x 64 channels), 65536 pixels per row.

Histogram (256 bins per row) is computed by three concurrent engine lanes,
each owning a bin range:
  - ACT lane  (bins 0..ACT_BINS-1): count(q >= b) via Sigmoid(60*(q-b+.5))
    with accumulate; exact for integer codes. hist by differencing.
  - DVE lane  (ACT_BINS..ACT_BINS+DVE_BINS-1): fused is_equal+accum (1x).
  - PE lane   (rest): codes DMA-transposed to pixel-major chunks; DVE builds
    is_equal indicators at 4x; PE ones-stationary matmuls reduce over the
    128-pixel partition dim into per-row counts, PSUM-accumulated across the
    row (6 bins resident per group pass, qT restreamed per group via xbar).
Then: first-index argmax -> dominant bin; phase 4 builds the attention mask
from raw x and applies folded conv1/bn1/relu, conv2/bn2/sigmoid, out = x*s.
"""
import sys
import numpy as np

sys.path.insert(0, "/opt/trn_rl_repo")

_CACHE = {}

BN_EPS = 1e-5
NBINS = 256
SCALE = float(np.float32(256.0 / 255.0))
SIGK = 60.0

ACT_BINS = 78     # bins [0, 78) counted on ACT via cge diffs (ops cge[1..78])
DVE_BINS = 17     # bins [78, 95) on DVE fused is_equal+accum
PE_START = ACT_BINS + DVE_BINS            # 95
PE_BINS = NBINS - PE_START                # 161
PSUM_GROUP = 6    # PE-lane bins resident in PSUM per group pass


def _build(hw=65536, dbg=False):
    from contextlib import ExitStack
    import concourse.bass as bass
    import concourse.tile as tile
    from concourse import bacc, mybir
    from concourse.tile import add_dep_helper

    f32 = mybir.dt.float32
    bf16 = mybir.dt.bfloat16
    i32 = mybir.dt.int32
    Alu = mybir.AluOpType
    Act = mybir.ActivationFunctionType

    P = 128
    CNK = 8192                 # counting chunk (pixels)
    n_cnk = hw // CNK          # 8
    P1C = 1024                 # phase-1 sub-chunk
    p1_per_cnk = CNK // P1C    # 8
    IND = 4096                 # indicator sub-chunk (2 per counting chunk)
    ACTC = hw // 2             # ACT accum chunk (2 per bin)
    p4_chunk = 2048
    n_p4 = hw // p4_chunk
    n_sub = p4_chunk // 512

    pe_bins = list(range(PE_START, NBINS))
    groups = [pe_bins[i:i + PSUM_GROUP] for i in range(0, len(pe_bins), PSUM_GROUP)]
    n_groups = len(groups)

    nc = bacc.Bacc(None, target_bir_lowering=False, debug=False)

    x_dram = nc.dram_tensor("x", [P, hw], f32, kind="ExternalInput")
    w1_dram = nc.dram_tensor("w1blk", [128, 128], f32, kind="ExternalInput")
    b1_dram = nc.dram_tensor("b1r", [128, 1], f32, kind="ExternalInput")
    w2_dram = nc.dram_tensor("w2blk", [128, 2], f32, kind="ExternalInput")
    b2_dram = nc.dram_tensor("b2r", [2, 1], f32, kind="ExternalInput")
    sel_dram = nc.dram_tensor("sel2", [2, 128], f32, kind="ExternalInput")
    iota_dram = nc.dram_tensor("iota", [128, NBINS], f32, kind="ExternalInput")
    bias_dram = nc.dram_tensor("biast", [128, NBINS], f32, kind="ExternalInput")
    ones_dram = nc.dram_tensor("onesw", [128, 128], bf16, kind="ExternalInput")
    out_dram = nc.dram_tensor("out", [P, hw], f32, kind="ExternalOutput")
    if dbg:
        hist_dram = nc.dram_tensor("hist_dbg", [P, NBINS], f32, kind="ExternalOutput")
        dom_dram = nc.dram_tensor("dom_dbg", [P, 1], f32, kind="ExternalOutput")

    with tile.TileContext(nc) as tc, ExitStack() as top:
        const = top.enter_context(tc.tile_pool(name="const", bufs=1))

        w1t = const.tile([128, 128], f32)
        nc.sync.dma_start(w1t[:], w1_dram.ap())
        b1t = const.tile([128, 1], f32)
        nc.sync.dma_start(b1t[:], b1_dram.ap())
        w2t = const.tile([128, 2], f32)
        nc.sync.dma_start(w2t[:], w2_dram.ap())
        b2t = const.tile([2, 1], f32)
        nc.sync.dma_start(b2t[:], b2_dram.ap())
        selt = const.tile([2, 128], f32)
        nc.sync.dma_start(selt[:], sel_dram.ap())
        iotat = const.tile([128, NBINS], f32)
        nc.sync.dma_start(iotat[:], iota_dram.ap())
        biastt = const.tile([128, NBINS], f32)
        nc.sync.dma_start(biastt[:], bias_dram.ap())
        onest = const.tile([128, 128], bf16)
        nc.sync.dma_start(onest[:], ones_dram.ap())

        cge_parts = const.tile([128, ACT_BINS * 2], f32)
        cnt_d = const.tile([128, DVE_BINS * n_cnk], f32)
        hist = const.tile([128, NBINS], f32)
        cge = const.tile([128, ACT_BINS], f32)
        t1 = const.tile([128, NBINS], f32)
        mx = const.tile([128, 1], f32)
        dom = const.tile([128, 1], f32)
        domp1 = const.tile([128, 1], f32)
        trash_a = const.tile([128, 8], bf16)   # ACT stride-0 discard out
        trash_d = const.tile([128, 8], bf16)   # DVE stride-0 discard out

        # DVE engine-order chain (nosync deps pin the stream order)
        prev_dve = [None]

        def chain(v):
            if prev_dve[0] is not None:
                add_dep_helper(v.ins, prev_dve[0], sync=False,
                               reason="dve stream order")
            prev_dve[0] = v.ins

        with ExitStack() as mid:
            qpool = mid.enter_context(tc.tile_pool(name="qpool", bufs=1))
            qcodes = qpool.tile([P, hw], bf16)
            qtp = mid.enter_context(tc.tile_pool(name="qtp", bufs=2))
            indp = mid.enter_context(tc.tile_pool(name="indp", bufs=2))
            redp = mid.enter_context(tc.tile_pool(name="redp", bufs=1))
            p1x = mid.enter_context(tc.tile_pool(name="p1x", bufs=2))
            p1s = mid.enter_context(tc.tile_pool(name="p1s", bufs=1))
            psp = mid.enter_context(tc.tile_pool(name="psp", bufs=1, space="PSUM"))

            def emit_p1(c):
                """Exact codes q = trunc(x*S) for p1-chunk c (P1C pixels).
                y = x*S; i = rne_i32(y); f = f32(i); q = f - (f > y)."""
                sl = slice(c * P1C, (c + 1) * P1C)
                xt = p1x.tile([P, P1C], f32, tag="xt")
                nc.scalar.dma_start(xt[:], x_dram.ap()[:, sl])
                yt = p1s.tile([P, P1C], f32, tag="yt")
                chain(nc.vector.tensor_scalar(out=yt[:], in0=xt[:], scalar1=SCALE,
                                              scalar2=None, op0=Alu.mult))
                it_ = p1s.tile([P, P1C], i32, tag="it")
                chain(nc.vector.tensor_copy(it_[:], yt[:]))
                ft = p1s.tile([P, P1C], f32, tag="ft")
                chain(nc.vector.tensor_copy(ft[:], it_[:]))
                chain(nc.vector.tensor_tensor(out=xt[:], in0=ft[:], in1=yt[:],
                                              op=Alu.is_gt))
                chain(nc.vector.tensor_tensor(out=qcodes[:, sl], in0=ft[:],
                                              in1=xt[:], op=Alu.subtract))

            def emit_act(b, h):
                """cge accumulation: count(q >= b) over half h."""
                sl = slice(h * ACTC, (h + 1) * ACTC)
                col = (b - 1) * 2 + h
                nc.scalar.activation(
                    out=trash_a[:, 0:1].broadcast_to([P, ACTC]),
                    in_=qcodes[:, sl], func=Act.Sigmoid,
                    bias=biastt[:, b:b + 1], scale=SIGK,
                    accum_out=cge_parts[:, col:col + 1])

            def emit_fused(b, c):
                """DVE-lane fused count of bin b over counting chunk c."""
                sl = slice(c * CNK, (c + 1) * CNK)
                col = (b - ACT_BINS) * n_cnk + c
                chain(nc.vector.tensor_scalar(
                    out=trash_d[:, 0:1].broadcast_to([P, CNK]),
                    in0=qcodes[:, sl], scalar1=float(b), scalar2=None,
                    op0=Alu.is_equal, op1=Alu.add,
                    accum_out=cnt_d[:, col:col + 1]))

            fused_q = [(b, c) for b in range(ACT_BINS, ACT_BINS + DVE_BINS)
                       for c in range(n_cnk)]
            fused_i = [0]
            total_slots = max(1, (n_groups - 1) * n_cnk)

            def drain_fused(slots_done):
                tgt = min(len(fused_q), int(len(fused_q) * slots_done / total_slots) + 1)
                while fused_i[0] < tgt:
                    b, c = fused_q[fused_i[0]]
                    emit_fused(b, c)
                    fused_i[0] += 1

            # ---- merged counting loop ----
            for g, bins in enumerate(groups):
                psums = {}
                for bi, b in enumerate(bins):
                    pst = psp.tile([P, 512], f32, tag=f"ps{bi}")
                    psums[b] = pst
                for j in range(n_cnk):
                    if g == 0:
                        for c in range(j * p1_per_cnk, (j + 1) * p1_per_cnk):
                            emit_p1(c)
                    qt = qtp.tile([P, CNK], bf16, tag="qt")
                    nc.sync.dma_start(qt[:].rearrange("p (s r) -> p s r", r=128),
                                      qcodes[:, j * CNK:(j + 1) * CNK],
                                      transpose=True)
                    for b in bins:
                        for h in range(CNK // IND):
                            ind = indp.tile([P, IND], bf16, tag="ind")
                            chain(nc.vector.tensor_scalar(
                                out=ind[:], in0=qt[:, h * IND:(h + 1) * IND],
                                scalar1=float(b), scalar2=None, op0=Alu.is_equal))
                            base = j * CNK + h * IND  # for start/stop bookkeeping
                            for k in range(IND // 512):
                                first = (j == 0 and h == 0 and k == 0)
                                last = (j == n_cnk - 1 and h == CNK // IND - 1
                                        and k == IND // 512 - 1)
                                nc.tensor.matmul(psums[b][:], onest[:],
                                                 ind[:, k * 512:(k + 1) * 512],
                                                 start=first, stop=last)
                    if g == 0 and j == 3:
                        for b in range(1, ACT_BINS + 1):
                            emit_act(b, 0)
                    if g == 0 and j == n_cnk - 1:
                        for b in range(1, ACT_BINS + 1):
                            emit_act(b, 1)
                    if g > 0:
                        drain_fused((g - 1) * n_cnk + j + 1)
                # group end: reduce psums [128,(4c,128r)] -> [128,128], assemble
                red = redp.tile([P, PSUM_GROUP * 128], f32, tag="red")
                for bi, b in enumerate(bins):
                    pv = psums[b][:].rearrange("p (c r) -> p r c", r=128)
                    chain(nc.vector.tensor_reduce(
                        out=red[:, bi * 128:(bi + 1) * 128], in_=pv,
                        axis=mybir.AxisListType.X, op=Alu.add))
                for bi, b in enumerate(bins):
                    nc.sync.dma_start(hist[:, b:b + 1],
                                      red[0:1, bi * 128:(bi + 1) * 128])

            # ---- finish ACT-lane and DVE-lane histogram columns ----
            cgev = cge_parts[:].rearrange("p (b h) -> p b h", h=2)
            chain(nc.vector.tensor_reduce(out=cge[:], in_=cgev,
                                          axis=mybir.AxisListType.X, op=Alu.add))
            # hist[0] = hw - cge[1]; hist[b] = cge[b] - cge[b+1] (cge col b-1)
            chain(nc.vector.tensor_scalar(out=hist[:, 0:1], in0=cge[:, 0:1],
                                          scalar1=-1.0, scalar2=float(hw),
                                          op0=Alu.mult, op1=Alu.add))
            chain(nc.vector.tensor_tensor(out=hist[:, 1:ACT_BINS],
                                          in0=cge[:, 0:ACT_BINS - 1],
                                          in1=cge[:, 1:ACT_BINS], op=Alu.subtract))
            cdv = cnt_d[:].rearrange("p (b c) -> p b c", c=n_cnk)
            chain(nc.vector.tensor_reduce(out=hist[:, ACT_BINS:PE_START], in_=cdv,
                                          axis=mybir.AxisListType.X, op=Alu.add))

        # ---- Phase 3: first-index argmax over hist ----
        nc.vector.tensor_reduce(out=mx[:], in_=hist[:], axis=mybir.AxisListType.X,
                                op=Alu.max)
        nc.vector.tensor_scalar(out=t1[:], in0=hist[:], scalar1=mx[:],
                                scalar2=1.0e6, op0=Alu.not_equal, op1=Alu.mult)
        nc.vector.tensor_tensor(out=t1[:], in0=t1[:], in1=iotat[:], op=Alu.add)
        nc.vector.tensor_reduce(out=dom[:], in_=t1[:], axis=mybir.AxisListType.X,
                                op=Alu.min)
        nc.vector.tensor_scalar(out=domp1[:], in0=dom[:], scalar1=1.0, scalar2=None,
                                op0=Alu.add)
        if dbg:
            nc.sync.dma_start(hist_dram.ap(), hist[:])
            nc.sync.dma_start(dom_dram.ap(), dom[:])

        # ---- Phase 4: mask, convs, output ----
        with ExitStack() as p4:
            px = p4.enter_context(tc.tile_pool(name="px", bufs=3))
            pw = p4.enter_context(tc.tile_pool(name="pw", bufs=2))
            pz = p4.enter_context(tc.tile_pool(name="pz", bufs=2))
            pout = p4.enter_context(tc.tile_pool(name="pout", bufs=3))
            ps_z = p4.enter_context(tc.tile_pool(name="ps_z", bufs=3, space="PSUM"))
            ps_s = p4.enter_context(tc.tile_pool(name="ps_s", bufs=2, space="PSUM"))
            ps_b = p4.enter_context(tc.tile_pool(name="ps_b", bufs=2, space="PSUM"))

            for j in range(n_p4):
                sl = slice(j * p4_chunk, (j + 1) * p4_chunk)
                xt = px.tile([P, p4_chunk], f32, tag="xt")
                nc.scalar.dma_start(xt[:], x_dram.ap()[:, sl])
                ga = pw.tile([P, p4_chunk], bf16, tag="ga")
                nc.vector.tensor_scalar(out=ga[:], in0=xt[:], scalar1=dom[:],
                                        scalar2=None, op0=Alu.is_ge)
                gb = pw.tile([P, p4_chunk], bf16, tag="gb")
                nc.vector.tensor_scalar(out=gb[:], in0=xt[:], scalar1=domp1[:],
                                        scalar2=None, op0=Alu.is_ge)
                mt = pw.tile([P, p4_chunk], f32, tag="mt")
                nc.vector.tensor_tensor(out=mt[:], in0=ga[:], in1=gb[:],
                                        op=Alu.subtract)
                zt = pz.tile([P, p4_chunk], f32, tag="zt")
                st = pz.tile([2, p4_chunk], f32, tag="st")
                ot = pout.tile([P, p4_chunk], f32, tag="ot")
                for k in range(n_sub):
                    ssl = slice(k * 512, (k + 1) * 512)
                    zp = ps_z.tile([128, 512], f32, tag="zp")
                    nc.tensor.matmul(zp[:], w1t[:], mt[:, ssl], start=True, stop=True)
                    nc.scalar.activation(out=zt[:, ssl], in_=zp[:], func=Act.Relu,
                                         bias=b1t[:], scale=1.0)
                    sp = ps_s.tile([2, 512], f32, tag="sp")
                    nc.tensor.matmul(sp[:], w2t[:], zt[:, ssl], start=True, stop=True)
                    nc.scalar.activation(out=st[:, ssl], in_=sp[:], func=Act.Sigmoid,
                                         bias=b2t[:], scale=1.0)
                    bp = ps_b.tile([128, 512], f32, tag="bp")
                    nc.tensor.matmul(bp[:], selt[:], st[:, ssl], start=True, stop=True)
                    nc.vector.tensor_tensor(out=ot[:, ssl], in0=xt[:, ssl], in1=bp[:],
                                            op=Alu.mult)
                nc.scalar.dma_start(out_dram.ap()[:, sl], ot[:])

    if not nc.is_finalized():
        nc.finalize()
    return nc


def _host_constants(conv1_w, conv1_b, bn1_gamma, bn1_beta, bn1_mean, bn1_var,
                    conv2_w, conv2_b, bn2_gamma, bn2_beta, bn2_mean, bn2_var):
    """Fold BN into conv weights (float64, cast f32) and build layout blocks.

    Phase 4 computes m = [dom <= x < dom+1] = 1 - att, so conv1 is applied with
    negated weights and bias shifted by the row sums: W1'(1-m) = (W1'*1 - W1'*m).
    """
    import ml_dtypes
    C = conv1_w.shape[0]
    inv1 = (bn1_gamma.astype(np.float64)
            / np.sqrt(bn1_var.astype(np.float64) + BN_EPS))
    w1f = conv1_w.astype(np.float64) * inv1[:, None]          # [o, c]
    b1f = (conv1_b.astype(np.float64) * inv1
           + bn1_beta.astype(np.float64)
           - bn1_mean.astype(np.float64) * inv1)              # [o]
    # att = 1 - m fold
    b1n = b1f + w1f.sum(axis=1)
    w1n = -w1f

    inv2 = (bn2_gamma.astype(np.float64)
            / np.sqrt(bn2_var.astype(np.float64) + BN_EPS))
    w2f = conv2_w[0].astype(np.float64) * inv2[0]             # [c]
    b2f = (conv2_b.astype(np.float64) * inv2
           + bn2_beta.astype(np.float64)
           - bn2_mean.astype(np.float64) * inv2)              # [1]

    w1blk = np.zeros((128, 128), np.float32)
    w1t = w1n.T.astype(np.float32)                            # [c, o]
    w1blk[:C, :C] = w1t
    w1blk[C:, C:] = w1t
    b1r = np.tile(b1n.astype(np.float32), 2).reshape(128, 1)

    w2blk = np.zeros((128, 2), np.float32)
    w2blk[:C, 0] = w2f.astype(np.float32)
    w2blk[C:, 1] = w2f.astype(np.float32)
    b2r = np.full((2, 1), b2f[0], np.float32)

    sel2 = np.zeros((2, 128), np.float32)
    sel2[0, :C] = 1.0
    sel2[1, C:] = 1.0

    iota = np.tile(np.arange(NBINS, dtype=np.float32), (128, 1))
    biast = np.zeros((128, NBINS), np.float32)
    for b in range(1, NBINS):
        biast[:, b] = -SIGK * (b - 0.5)
    onesw = np.ones((128, 128), dtype=ml_dtypes.bfloat16)
    return dict(w1blk=w1blk, b1r=b1r, w2blk=w2blk, b2r=b2r, sel2=sel2, iota=iota,
                biast=biast, onesw=onesw)


def _run(x, conv1_w, conv1_b, bn1_gamma, bn1_beta, bn1_mean, bn1_var,
         conv2_w, conv2_b, bn2_gamma, bn2_beta, bn2_mean, bn2_var,
         trace=False):
    from concourse.bass_utils import run_bass_kernel_spmd

    x = np.asarray(x, np.float32)
    B, C, H, W = x.shape
    hw = H * W
    n_cores = 8
    bpc = B // n_cores  # batches per core

    key = ("nc_v2", hw)
    if key not in _CACHE:
        _CACHE[key] = _build(hw=hw)
    nc = _CACHE[key]

    consts = _host_constants(
        np.asarray(conv1_w), np.asarray(conv1_b), np.asarray(bn1_gamma),
        np.asarray(bn1_beta), np.asarray(bn1_mean), np.asarray(bn1_var),
        np.asarray(conv2_w), np.asarray(conv2_b), np.asarray(bn2_gamma),
        np.asarray(bn2_beta), np.asarray(bn2_mean), np.asarray(bn2_var))

    xs = x.reshape(n_cores, bpc * C, hw)
    in_maps = [dict(x=np.ascontiguousarray(xs[i]), **consts) for i in range(n_cores)]

    res = run_bass_kernel_spmd(nc, in_maps, core_ids=list(range(n_cores)),
                               trace=trace)
    outs = [res.results[i]["out"].reshape(bpc, C, H, W) for i in range(n_cores)]
    return np.concatenate(outs, axis=0).astype(np.float32), res


def kernel(**inputs):
    out, _ = _run(**inputs)
    return out
